# revision 63
# baseline (speedup 1.0000x reference)
"""DirectionalSelfAttention Trainium2 kernel (8 NeuronCores).

Sharding: core c handles (batch b = c//2, head-group g = c%2) -> 8 heads each.
Per-core device kernel (all matmuls bf16, fp32 PSUM accumulation):
  QKV:  Q^T/K^T packs [128=2 heads x 64, T] and V natural [T, 64] per head
        (with a ones column -> softmax denominator rides the PV matmul).
  Attn: S^T tiles [128 k, 512 q] = K^T.T @ Q^T (K=64 contraction,
        2-head tile_position row packing), exp on ScalarE over paired
        [128,1024] PSUM tiles, causal/anti-causal tile skipping + bf16
        mask multiply on diagonal tiles, O_aug^T = V_aug.T @ P^T.
  Norm: V carries 64 ones-columns so the denominator lands replicated on
        PSUM partitions 64-127; one fast-approx reciprocal per (qb,p)
        covers both heads, DVE multiply -> normalized O^T packs.
  Proj: y_partial[T, 1024] = O_loc @ proj_w[g*512:(g+1)*512] -> DRAM.
Host: builds x^T/weight slices in bf16, sums the two per-batch partials,
adds proj_b.
"""

import math
import os
import sys
import types

import numpy as np
import ml_dtypes

import concourse.bass as bass
import concourse.tile as tile
from concourse import bacc, mybir
from concourse.bass_utils import run_bass_kernel_spmd
from concourse.vector_clock import ScopedClock

N_CORES = 8
B, T, C = 4, 2048, 1024
H, D = 16, 64
H_LOC = 8          # heads per core
C_LOC = 512        # channels per core (head-group)
QB = 512           # q-block (matmul moving free dim)
KC = 128           # k-chunk (PSUM partition dim)
N_QB = T // QB     # 4
N_KC = T // KC     # 16
N_CC = C // 128    # 8 contraction chunks for the projections
SCALE = 1.0 / math.sqrt(D)

BF16 = mybir.dt.bfloat16
F32 = mybir.dt.float32

LAST_RESULT = None  # BassKernelResults of the most recent run (for test.py)


def _patch_tile_tail_drain():
    """This walrus build only encodes a limited number of sync-waits per
    instruction; Tile's kernel-tail drain aggregates one wait per
    outstanding proc and overflows that. Spread the waits across SP NOPs."""
    if getattr(tile.TileContext, "_tail_drain_patched", False):
        return

    def _drain_and_barrier(self, tick_clock, wait_clock):
        probe = self.nc.sync.nop(nofuse=True)
        wait_clock.add_sem_waits(
            probe.ins, ScopedClock({None: tick_clock.global_clock})
        )
        si = probe.ins.sync_info
        waits = list(si.on_wait) if si and si.on_wait else []
        if si:
            si.on_wait = waits[:1]
        for w in waits[1:]:
            n = self.nc.sync.nop(nofuse=True)
            n.ins.sync_info = mybir.SyncInfo(on_wait=[w], on_update=[])
        self.nc.sync.drain()
        self.nc.all_engine_barrier()
        assert self.sems is not None
        popped = self.nc._tile_sem_poison_stack.pop()
        assert popped is self._sem_poison
        self.nc.clear_and_free_semaphores(list(self.sems.allocated().values()))
        self.nc.all_engine_barrier()

    tile.TileContext._drain_and_barrier = _drain_and_barrier
    tile.TileContext._tail_drain_patched = True


def _install_ntff_shim():
    """antenv.axon_hooks is absent in this image; recreate it so
    run_bass_kernel_spmd(trace=True) can NTFF-profile under axon."""
    if "antenv.axon_hooks" in sys.modules:
        return
    try:
        from trn_agent_boot.trn_boot import _ntff_profile_via_ctypes

        hook = _ntff_profile_via_ctypes("/opt/axon/libaxon_pjrt.so")
    except Exception:
        hook = None
    mod = types.ModuleType("antenv.axon_hooks")
    state = [hook]
    mod.set_axon_ntff_profile_hook = lambda h: state.__setitem__(0, h)
    mod.get_axon_ntff_profile_hook = lambda: state[0]
    sys.modules["antenv.axon_hooks"] = mod
    try:
        import antenv

        antenv.axon_hooks = mod
    except Exception:
        pass


def _allowed_kcs(qb, anti):
    """k-chunks contributing to q-block qb, ascending; always even count."""
    if anti:
        return list(range(4 * qb, N_KC))
    return list(range(0, 4 * qb + 4))


def _build_masks(anti):
    """Diagonal-tile masks [4, 128, 512] bf16.

    Variant r (= kc - 4*qb) allows, at (k-partition kp, q-free qf):
      causal:      qf >= kp + 128*r
      anti-causal: qf <= kp + 128*r
    """
    kp = np.arange(KC)[:, None]
    qf = np.arange(QB)[None, :]
    ms = []
    for r in range(4):
        if anti:
            m = (qf <= kp + 128 * r)
        else:
            m = (qf >= kp + 128 * r)
        ms.append(m.astype(np.float32))
    return np.stack(ms).astype(ml_dtypes.bfloat16)


def _build_program(anti, has_bqk, has_bv):
    nc = bacc.Bacc("TRN2", target_bir_lowering=False, debug=False,
                   num_devices=N_CORES)

    xt_d = nc.dram_tensor("xt", [C, T], BF16, kind="ExternalInput").ap()
    wq_d = nc.dram_tensor("wq", [C, C_LOC], BF16, kind="ExternalInput").ap()
    wk_d = nc.dram_tensor("wk", [C, C_LOC], BF16, kind="ExternalInput").ap()
    wv_d = nc.dram_tensor("wv", [C, C_LOC], BF16, kind="ExternalInput").ap()
    wp_d = nc.dram_tensor("wp", [C_LOC, C], BF16, kind="ExternalInput").ap()
    mk_d = nc.dram_tensor("masks", [4, KC, QB], BF16,
                          kind="ExternalInput").ap()
    if has_bqk:
        bq_d = nc.dram_tensor("bq", [128, 4], F32, kind="ExternalInput").ap()
        bk_d = nc.dram_tensor("bk", [128, 4], F32, kind="ExternalInput").ap()
    if has_bv:
        bv_d = nc.dram_tensor("bv", [64, 8], F32, kind="ExternalInput").ap()
    y_d = nc.dram_tensor("y", [T, C], F32, kind="ExternalOutput").ap()

    with tile.TileContext(nc) as tc:
        with (
            tc.tile_pool(name="persist", bufs=1) as persist,
            tc.tile_pool(name="pt", bufs=7) as pt_pool,
            tc.tile_pool(name="ysb", bufs=3) as y_pool,
            tc.tile_pool(name="rbc", bufs=3) as rb_pool,
            tc.tile_pool(name="otmp", bufs=2) as ot_pool,
            tc.tile_pool(name="ps_mm", bufs=2, space="PSUM") as ps_mm,
            tc.tile_pool(name="ps_s", bufs=2, space="PSUM") as ps_s,
            tc.tile_pool(name="ps_o", bufs=2, space="PSUM") as ps_o,
        ):
            xt = persist.tile([128, N_CC, T], BF16, tag="xt")
            wq = persist.tile([128, N_CC, C_LOC], BF16, tag="wq")
            wk = persist.tile([128, N_CC, C_LOC], BF16, tag="wk")
            wv = persist.tile([128, N_CC, C_LOC], BF16, tag="wv")
            wp = persist.tile([128, 4, C], BF16, tag="wp")
            mk = persist.tile([128, 4, QB], BF16, tag="mk")
            qt = persist.tile([128, 4, T], BF16, tag="qt")
            kt = persist.tile([128, 4, T], BF16, tag="kt")
            # V_aug per (kc, head): cols 0-63 = V, cols 64-127 = ones, so the
            # PV matmul leaves the softmax denominator replicated on PSUM
            # partitions 64..127 (free partition-broadcast).
            va = persist.tile([128, N_KC, H_LOC, 128], BF16, tag="va")
            oa = persist.tile([128, 4, T], BF16, tag="oa")

            # ---- loads: wq/x^T interleaved per contraction chunk so the
            # first Q-projection matmuls start as soon as chunk 0 lands ----
            wq_src = wq_d.rearrange("(cc p) n -> p cc n", p=128)
            xt_src = xt_d.rearrange("(cc p) t -> p cc t", p=128)
            for cc in range(N_CC):
                nc.sync.dma_start(wq[:, cc, :], wq_src[:, cc, :])
                nc.sync.dma_start(xt[:, cc, :], xt_src[:, cc, :])
            nc.sync.dma_start(wk[:], wk_d.rearrange("(cc p) n -> p cc n", p=128))
            nc.sync.dma_start(mk[:], mk_d.rearrange("r p q -> p r q"))
            nc.sync.dma_start(wv[:], wv_d.rearrange("(cc p) n -> p cc n", p=128))
            nc.sync.dma_start(wp[:], wp_d.rearrange("(p j) n -> j p n", j=128))

            # ---- PE pre-warmer: dummy matmuls keep the PE HAM activity
            # monitor busy through the DMA prologue so real matmuls start at
            # the full 2.4 GHz clock instead of the throttled 1.2 GHz ----
            warm = persist.tile([128, QB], BF16, tag="warm")
            nc.gpsimd.memset(warm[:], 0.0)
            ps_w = ps_mm.tile([128, QB], F32, tag="mm", name="warmps")
            for _ in range(8):
                nc.tensor.matmul(ps_w[:], warm[:, 0:128], warm[:],
                                 start=True, stop=True)
            if has_bqk:
                bq = persist.tile([128, 4], F32, tag="bq")
                bk = persist.tile([128, 4], F32, tag="bk")
                nc.sync.dma_start(bq[:], bq_d)
                nc.sync.dma_start(bk[:], bk_d)
            if has_bv:
                bv = persist.tile([64, 8], F32, tag="bv")
                nc.sync.dma_start(bv[:], bv_d)
            # ones block for the denominator
            nc.gpsimd.memset(va[:, :, :, 64:128], 1.0)

            # ---- emission helpers ----
            def qk_proj_pack(w_tile, dst, bias_tile, p):
                for qb in range(N_QB):
                    ps = ps_mm.tile([128, QB], F32, tag="mm", name="mm")
                    for cc in range(N_CC):
                        nc.tensor.matmul(
                            ps[:],
                            w_tile[:, cc, p * 128:(p + 1) * 128],
                            xt[:, cc, qb * QB:(qb + 1) * QB],
                            start=(cc == 0), stop=(cc == N_CC - 1),
                        )
                    dst_ap = dst[:, p, qb * QB:(qb + 1) * QB]
                    if bias_tile is not None:
                        nc.scalar.activation(
                            dst_ap, ps[:],
                            mybir.ActivationFunctionType.Identity,
                            bias=bias_tile[:, p:p + 1],
                        )
                    else:
                        nc.vector.tensor_copy(dst_ap, ps[:])

            def v_group(tcs):
                for tc_i in tcs:
                    ps = ps_mm.tile([128, QB], F32, tag="mm", name="mm")
                    for cc in range(N_CC):
                        nc.tensor.matmul(
                            ps[:],
                            xt[:, cc, tc_i * 128:(tc_i + 1) * 128],
                            wv[:, cc, :],
                            start=(cc == 0), stop=(cc == N_CC - 1),
                        )
                    nc.vector.tensor_copy(
                        va[:, tc_i, :, 0:64],
                        ps[:].rearrange("p (l d) -> p l d", d=64),
                    )

            def attn(qb, p, last=False):
                """Generator: yields after each k-chunk so two (qb,p)
                streams can interleave their S/exp/PV chains — one stream's
                matmuls fill the other's exp latency."""
                kcs = _allowed_kcs(qb, anti)
                nkc = len(kcs)
                o_ps = [ps_o.tile([128, QB], F32, tag="o", name=f"o{m}")
                        for m in (0, 1)]
                pending = []
                for ki, kc in enumerate(kcs):
                    # one s tile holds both heads' S^T for this k-chunk:
                    # head A in cols 0:512, head B in cols 512:1024. Both S
                    # matmuls are gated by the same slot release, so the B
                    # matmul (row group 64) packs concurrently behind A.
                    # On diagonal tiles only columns [lo:hi) are unmasked —
                    # S/exp/mask/PV are all restricted to that range.
                    diag = (kc >= 4 * qb) if not anti else (kc < 4 * qb + 4)
                    r = kc - 4 * qb
                    if diag and not anti:
                        lo, hi = 128 * r, QB
                    elif diag:
                        lo, hi = 0, 128 * (r + 1)
                    else:
                        lo, hi = 0, QB
                    s_ps = ps_s.tile([128, 2 * QB], F32, tag="s", name="s")
                    s3 = s_ps.rearrange("p (m q) -> p m q", m=2)
                    for m in (0, 1):
                        sl = slice(m * 64, (m + 1) * 64)
                        nc.tensor.matmul(
                            s3[:, m, lo:hi],
                            kt[sl, p, kc * KC:(kc + 1) * KC],
                            qt[sl, p, qb * QB + lo:qb * QB + hi],
                            start=True, stop=True,
                            tile_position=(m * 64, 0),
                        )
                    pt = pt_pool.tile([128, 2 * QB], BF16, tag="pt", name="pt")
                    pt3 = pt.rearrange("p (m q) -> p m q", m=2)
                    nc.scalar.activation(
                        pt3[:, :, lo:hi], s3[:, :, lo:hi],
                        mybir.ActivationFunctionType.Exp,
                        scale=SCALE,
                    )
                    if diag:
                        for m in (0, 1):
                            nc.vector.tensor_mul(
                                pt3[:, m, lo:hi],
                                pt3[:, m, lo:hi],
                                mk[:, r, lo:hi],
                            )
                    # software-pipeline by two k-chunks: emit PV lagging
                    # the S matmuls. The PE queue is strict FIFO, so a PV
                    # waiting on its exp (or, at stream start, on the o-slot
                    # release by the previous stream's normalize) would
                    # otherwise block the already-ready next S matmuls.
                    pending.append((pt3, lo, hi, kc, ki))
                    if len(pending) > 3:
                        _pt3, _lo, _hi, _kc, _ki = pending.pop(0)
                        for m in (0, 1):
                            nc.tensor.matmul(
                                o_ps[m][:, _lo:_hi],
                                va[:, _kc, 2 * p + m, :],
                                _pt3[:, m, _lo:_hi],
                                start=(_ki == 0), stop=False,
                            )
                    yield
                for _pt3, _lo, _hi, _kc, _ki in pending:
                    for m in (0, 1):
                        nc.tensor.matmul(
                            o_ps[m][:, _lo:_hi],
                            va[:, _kc, 2 * p + m, :],
                            _pt3[:, m, _lo:_hi],
                            start=(_ki == 0), stop=(_ki == nkc - 1),
                        )
                # normalize + store into O^T packs; one [128,512] reciprocal
                # covers both heads' denominators (the op is pass-dominated,
                # its cost doesn't depend on partition count)
                qsl = slice(qb * QB, (qb + 1) * QB)
                dn = rb_pool.tile([128, QB], F32, tag="dn", name="dn")
                rb = rb_pool.tile([128, QB], F32, tag="rb", name="rb")
                nc.vector.tensor_copy(dn[0:64, :], o_ps[0][64:128, :])
                nc.vector.tensor_copy(dn[64:128, :], o_ps[1][64:128, :])
                # NB: reciprocal_approx_fast silently misbehaves on partition
                # slices with base != 0 — only ever call it on full tiles.
                nc.vector.reciprocal_approx_fast(rb[:], dn[:])
                for m in (0, 1):
                    if m == 0:
                        dst = oa[0:64, p, qsl]
                        nc.vector.tensor_mul(dst, o_ps[m][0:64, :],
                                             rb[0:64, :])
                        if has_bv:
                            nc.vector.tensor_scalar_add(
                                dst, dst, bv[0:64, 2 * p:2 * p + 1]
                            )
                    elif last:
                        # final stream: write base-64 directly (DVE handles
                        # the cross-base in0) to keep the SBUF->SBUF DMA hop
                        # off the closing projection's critical path
                        dst = oa[64:128, p, qsl]
                        nc.vector.tensor_mul(dst, o_ps[m][0:64, :],
                                             rb[64:128, :])
                        if has_bv:
                            nc.vector.tensor_scalar_add(
                                dst, dst, bv[0:64, 2 * p + 1:2 * p + 2]
                            )
                    else:
                        ot = ot_pool.tile([64, QB], BF16, tag="ot", name="ot")
                        nc.vector.tensor_mul(ot[:], o_ps[m][0:64, :],
                                             rb[64:128, :])
                        if has_bv:
                            nc.vector.tensor_scalar_add(
                                ot[:], ot[:], bv[0:64, 2 * p + 1:2 * p + 2]
                            )
                        nc.sync.dma_start(oa[64:128, p, qsl], ot[:])

            def proj(qb):
                for tc_i in range(4 * qb, 4 * qb + 4):
                    for ob in range(2):
                        ps = ps_mm.tile([128, QB], F32, tag="mm", name="mm")
                        for p in range(4):
                            nc.tensor.matmul(
                                ps[:],
                                oa[:, p, tc_i * 128:(tc_i + 1) * 128],
                                wp[:, p, ob * QB:(ob + 1) * QB],
                                start=(p == 0), stop=(p == 3),
                            )
                        ysb = y_pool.tile([128, QB], F32, tag="y", name="y")
                        nc.vector.tensor_copy(ysb[:], ps[:])
                        nc.sync.dma_start(
                            y_d[tc_i * 128:(tc_i + 1) * 128,
                                ob * QB:(ob + 1) * QB],
                            ysb[:],
                        )

            # ---- interleaved emission: V/QK feed attention ASAP so the
            # ScalarE exp chain (the attention bottleneck) starts early ----
            def drive(gens):
                gens = list(gens)
                while gens:
                    for g in list(gens):
                        try:
                            next(g)
                        except StopIteration:
                            gens.remove(g)

            # p-major: pack p's attention (ScalarE-paced) starts right after
            # its Q/K projections; the next pack's QKV matmuls and the
            # previous q-block's projections fill the PE under it.
            qb_order = list(range(N_QB)) if not anti else list(range(N_QB - 1, -1, -1))
            for p in range(4):
                qk_proj_pack(wq, qt, bq if has_bqk else None, p)
                qk_proj_pack(wk, kt, bk if has_bqk else None, p)
                for qi, qb in enumerate(qb_order):
                    if p == 0:
                        v_group(range(4 * qb, 4 * qb + 4))
                    drive([attn(qb, p,
                                last=(p == 3 and qb == qb_order[-1]))])
                    if p == 3:
                        proj(qb)
    return nc


def kernel(x, direction, qkv_w, qkv_b, proj_w, proj_b):
    _patch_tile_tail_drain()
    trace = bool(os.environ.get("KERNEL_TRACE"))
    if trace:
        _install_ntff_shim()

    x = np.asarray(x, dtype=np.float32)
    qkv_w = np.asarray(qkv_w, dtype=np.float32)
    qkv_b = np.asarray(qkv_b, dtype=np.float32)
    proj_w = np.asarray(proj_w, dtype=np.float32)
    proj_b = np.asarray(proj_b, dtype=np.float32)
    dirn = int(np.asarray(direction))
    anti = dirn == 1

    bf = ml_dtypes.bfloat16
    has_bqk = bool(qkv_b[: 2 * C].any())
    has_bv = bool(qkv_b[2 * C:].any())

    masks = np.ascontiguousarray(_build_masks(anti))
    wqs = [np.ascontiguousarray(qkv_w[:, g * C_LOC:(g + 1) * C_LOC]).astype(bf)
           for g in range(2)]
    wks = [np.ascontiguousarray(
        qkv_w[:, C + g * C_LOC:C + (g + 1) * C_LOC]).astype(bf)
        for g in range(2)]
    wvs = [np.ascontiguousarray(
        qkv_w[:, 2 * C + g * C_LOC:2 * C + (g + 1) * C_LOC]).astype(bf)
        for g in range(2)]
    wps = [np.ascontiguousarray(proj_w[g * C_LOC:(g + 1) * C_LOC, :]).astype(bf)
           for g in range(2)]
    xts = [np.ascontiguousarray(x[b].T).astype(bf) for b in range(B)]

    in_maps = []
    for c in range(N_CORES):
        b, g = divmod(c, 2)
        im = {
            "xt": xts[b],
            "wq": wqs[g],
            "wk": wks[g],
            "wv": wvs[g],
            "wp": wps[g],
            "masks": masks,
        }
        if has_bqk:
            # bias value at (partition j, pack p) = qkv_b[region + g*512 + p*128 + j]
            bq = qkv_b[:C][g * C_LOC:(g + 1) * C_LOC].reshape(4, 128).T
            bk = qkv_b[C:2 * C][g * C_LOC:(g + 1) * C_LOC].reshape(4, 128).T
            im["bq"] = np.ascontiguousarray(bq).astype(np.float32)
            im["bk"] = np.ascontiguousarray(bk).astype(np.float32)
        if has_bv:
            # value at (partition d, local head l) = bv[g*512 + l*64 + d]
            bvv = qkv_b[2 * C:][g * C_LOC:(g + 1) * C_LOC].reshape(8, 64).T
            im["bv"] = np.ascontiguousarray(bvv).astype(np.float32)
        in_maps.append(im)

    nc = _build_program(anti, has_bqk, has_bv)
    nc.finalize()  # Bacc.compile(): wait splitting, regalloc, ACT table loads
    res = run_bass_kernel_spmd(
        nc, in_maps, core_ids=list(range(N_CORES)), trace=trace
    )
    global LAST_RESULT
    LAST_RESULT = res

    y = np.empty((B, T, C), dtype=np.float32)
    for b in range(B):
        y[b] = res.results[2 * b]["y"] + res.results[2 * b + 1]["y"]
    y += proj_b
    return y
